# revision 39
# baseline (speedup 1.0000x reference)
"""Trainium2 Bass kernel for nn_CrossAttention_26731876450618 (Lorentz/HyboNet
cross-attention, B=4 T=1024 H=16 HD=64, fp32).

Sharding: sequence-parallel over the 4096 (B*T) rows — core c owns rows
512c..512c+511 (batch c//2, t-half c%2). Q-side projection, attention, and the
output projection are fully local to each core; the K/V projections are
computed on each core's own y-rows and exchanged between batch-partner cores
(pairs {2b, 2b+1}) with one AllGather. The softmax is computed without
normalization (the Lorentz centroid divides by sqrt(|<ave,ave>|), which is
degree-1 homogeneous in ave, so the softmax denominator cancels; exp inputs
are bounded in [-3.3, 0] for this problem so no max-subtraction is needed).

Perf notes (v2):
- The ONLY scalar-engine functions used are Exp/Tanh/Square/Abs/Copy — all of
  which live in the same activation table set ("exp_and_others"), so exactly
  one ACT_TABLE_LOAD is issued for the whole kernel (the baseline's
  Sigmoid/Sqrt/Reciprocal mix caused ~58 table switches = 74us of stalls).
  sigmoid(x) = 0.5 + 0.5*tanh(x/2); sqrt/rsqrt are computed on the vector
  engine with a quake-style bit-trick seed + Newton iteration.
- Every matmul operand is bf16 (fp32 matmuls stream 4 cycles/column).
- Attention is software-pipelined: head h's score matmuls are emitted before
  head h-1's AV matmuls so the in-order PE queue never waits on the exps.
  Exps are batched in [128,1024] PSUM supertiles.
- sum_h ||cent_h||^2 is computed by matmul against zT^2 instead of per-head
  vector work; gathered V is staged to SBUF with 8 contiguous DMAs instead of
  128 strided ones.
"""

import numpy as np

import bass_rust
import concourse.bass as bass
import concourse.tile as tile
from concourse import mybir
from concourse.bass_utils import run_bass_kernel_spmd
from concourse.masks import make_identity

B, T, H, HD = 4, 1024, 16, 64
E, D = 1024, 1025
DA = D + 1          # augmented contraction dim (ones row for bias)
N_CORES = 8
RPC = 512           # rows per core
HP = H * (HD + 1)   # 1040: per-head [time, space64] packed
EPS = 1e-8
EPS_BITS = int(np.float32(EPS).view(np.int32))
MAGIC_P1 = 0x5F375A87   # quake rsqrt constant + 1 (for the ~(x>>1) form)

FP = mybir.dt.float32
I32 = mybir.dt.int32
BF = mybir.dt.bfloat16
DT_PROJ = BF
DT_ATTN = BF

AL = mybir.AluOpType
AF = mybir.ActivationFunctionType

KV_ELEMS = HP * RPC          # one assembled tensor (kT or v) per core
KV_CHUNK = 2 * KV_ELEMS      # kT + v


def _np_dt(dt):
    return np.dtype(mybir.dt.np(dt))


def _split_excess_waits(nc, keep=1):
    """This container's walrus encodes at most ONE sync-wait per instruction
    ("Too many sync wait commands" in setupSyncWait otherwise). Tile emits
    multi-wait instructions (notably the kernel-tail drain). Hoist all but the
    last wait of every instruction into standalone EventSemaphore instructions
    on the same engine queue immediately before it — identical semantics, one
    wait per instruction."""
    for fn in nc.m.functions:
        for bb in fn.blocks:
            insts = bb.instructions
            out = []
            changed = False
            for inst in insts:
                si = inst.sync_info
                waits = list(si.on_wait) if si is not None and si.on_wait else []
                if len(waits) > keep:
                    for k, w in enumerate(waits[:-keep]):
                        ev = bass_rust.InstEventSemaphore(
                            name=f"{inst.name}-hw{k}", ins=[], outs=[])
                        ev.engine = inst.engine
                        ev.sync_info = bass_rust.SyncInfo(on_wait=[w], on_update=[])
                        out.append(ev)
                    si.on_wait = waits[-keep:]
                    changed = True
                out.append(inst)
            if changed:
                insts[:] = out
                if len(bb.instructions) != len(out):
                    live = bb.instructions
                    while len(live):
                        live.pop()
                    live.extend(out)
                    assert len(bb.instructions) == len(out)


def build_nc():
    nc = bass.Bass(
        "TRN2", target_bir_lowering=False, debug=False, num_devices=N_CORES
    )

    xT = nc.dram_tensor("xT", [DA, RPC], DT_PROJ, kind="ExternalInput").ap()
    yT = nc.dram_tensor("yT", [DA, RPC], DT_PROJ, kind="ExternalInput").ap()
    WqA = nc.dram_tensor("WqA", [DA, D], DT_PROJ, kind="ExternalInput").ap()
    WkA = nc.dram_tensor("WkA", [DA, D], DT_PROJ, kind="ExternalInput").ap()
    WvA = nc.dram_tensor("WvA", [DA, D], DT_PROJ, kind="ExternalInput").ap()
    WoA = nc.dram_tensor("WoA", [DA, D], DT_PROJ, kind="ExternalInput").ap()
    esq = nc.dram_tensor("esq", [128, 1], FP, kind="ExternalInput").ap()
    esk = nc.dram_tensor("esk", [128, 1], FP, kind="ExternalInput").ap()
    esv = nc.dram_tensor("esv", [128, 1], FP, kind="ExternalInput").ap()
    eso = nc.dram_tensor("eso", [128, 1], FP, kind="ExternalInput").ap()
    sgn65 = nc.dram_tensor("sgn65", [65, 1], BF, kind="ExternalInput").ap()
    ones128 = nc.dram_tensor("ones128", [128, 1], BF, kind="ExternalInput").ap()
    ones0 = nc.dram_tensor("ones0", [128, 1], BF, kind="ExternalInput").ap()
    outT = nc.dram_tensor("outT", [D, RPC], FP, kind="ExternalOutput").ap()

    # d-chunking of the augmented contraction dim: 8 x 128 + 1 x 2
    DCH = [(i * 128, 128) for i in range(8)] + [(1024, 2)]
    # out-channel tiling of D=1025: 8 x 128 + 1 x 1
    CCH = [(i * 128, 128) for i in range(8)] + [(1024, 1)]

    with tile.TileContext(nc) as tc:
        with tc.tile_pool(name="persist", bufs=1) as persist, \
             tc.tile_pool(name="dram", bufs=1, space="DRAM") as dram:

            identity = persist.tile([128, 128], DT_ATTN)
            make_identity(nc, identity)
            sgn_sb = persist.tile([65, 1], BF)
            nc.sync.dma_start(out=sgn_sb, in_=sgn65)
            o128_sb = persist.tile([128, 1], BF)
            nc.sync.dma_start(out=o128_sb, in_=ones128)
            o0_sb = persist.tile([128, 1], BF)
            nc.sync.dma_start(out=o0_sb, in_=ones0)
            es_sb, es2_sb, esb_sb = {}, {}, {}
            for nm, ap in (("q", esq), ("k", esk), ("v", esv), ("o", eso)):
                t = persist.tile([128, 1], FP, name=f"es_{nm}")
                nc.sync.dma_start(out=t, in_=ap)
                es_sb[nm] = t
                t2 = persist.tile([128, 1], FP, name=f"es2_{nm}")
                nc.vector.tensor_scalar(t2, t, 0.5, None, op0=AL.mult)
                es2_sb[nm] = t2
                tb = persist.tile([128, 1], FP, name=f"esb_{nm}")
                nc.vector.tensor_scalar(tb, t2, 1.1, None, op0=AL.add)
                esb_sb[nm] = tb
            c025 = persist.tile([128, 1], FP)
            nc.vector.memset(c025, 0.25)
            onesr = persist.tile([1, 128], BF)
            nc.vector.memset(onesr, 1.0)

            # persistent attention outputs: cent-space chunks + squares + time
            zT = [persist.tile([128, RPC], BF, name=f"zT{j}") for j in range(8)]
            zsq = [persist.tile([128, RPC], BF, name=f"zsq{j}") for j in range(8)]
            z8 = persist.tile([2, RPC], BF)
            nc.vector.memset(z8, 1.0)  # row 0 overwritten with time later
            qT = [persist.tile([65, RPC], DT_ATTN, name=f"qT{h}") for h in range(16)]

            KH_ELEMS = KV_ELEMS // 2    # 8 heads' worth of kT
            k_send_h = [dram.tile([KH_ELEMS], DT_ATTN, name=f"ksend{x}")
                        for x in range(2)]
            v_send_d = dram.tile([KV_ELEMS], DT_ATTN)
            k_gath_h = [dram.tile([2 * KH_ELEMS], DT_ATTN, name=f"kgath{x}")
                        for x in range(2)]
            v_gath_d = dram.tile([2 * KV_ELEMS], DT_ATTN)
            kT_sends = [k_send_h[x].rearrange("(p f) -> p f", f=RPC)
                        for x in range(2)]
            v_send = v_send_d.rearrange("(p f) -> p f", f=HP)

            def gath_kT(x, j):
                # half x (heads 8x..8x+7), source core j of the pair
                return k_gath_h[x][j * KH_ELEMS:(j + 1) * KH_ELEMS].rearrange(
                    "(p f) -> p f", f=RPC)

            def gath_v(j):
                return v_gath_d[j * KV_ELEMS:(j + 1) * KV_ELEMS].rearrange(
                    "(p f) -> p f", f=HP)

            def load_chunks(pool, src, tag):
                ts = []
                for i, (d0, dn) in enumerate(DCH):
                    t = pool.tile([dn, RPC], DT_PROJ, name=f"{tag}{i}", tag=f"{tag}{i}")
                    nc.sync.dma_start(out=t, in_=src[d0:d0 + dn, :])
                    ts.append(t)
                return ts

            def load_w(pool, src, tag, bufs=1, eng=None):
                ts = []
                for i, (d0, dn) in enumerate(DCH):
                    t = pool.tile([dn, D], DT_PROJ, name=f"{tag}{i}", tag=f"{tag}{i}", bufs=bufs)
                    (eng or nc.sync).dma_start(out=t, in_=src[d0:d0 + dn, :])
                    ts.append(t)
                return ts

            def rsqrt(pool, x, tag, newton=1, bufs=4, out=None, final_mul=None,
                      eng=None):
                """Quake rsqrt: y ~= 1/sqrt(x) (x fp32 AP, > 0). With
                final_mul=m, the last op computes y*m instead (fused).
                Returns the output tile/AP written."""
                if eng is None:
                    eng = nc.vector
                shp = [x.shape[0], x.shape[-1]]
                y = pool.tile(shp, FP, name=f"y_{tag}", tag=f"y{tag}", bufs=bufs)
                eng.tensor_scalar(y.bitcast(I32), x.bitcast(I32), 1, -1,
                                  op0=AL.logical_shift_right,
                                  op1=AL.bitwise_xor)
                eng.tensor_scalar(y.bitcast(I32), y.bitcast(I32), MAGIC_P1,
                                  None, op0=AL.add)
                for it in range(newton):
                    last = it == newton - 1 and final_mul is None
                    t2 = pool.tile(shp, FP, name=f"t2_{tag}", tag=f"t2{tag}", bufs=bufs)
                    eng.tensor_mul(t2, y, y)
                    eng.scalar_tensor_tensor(t2, t2, -0.5, x,
                                             op0=AL.mult, op1=AL.mult)
                    dst = out if (last and out is not None) else y
                    eng.scalar_tensor_tensor(dst, t2, 1.5, y,
                                             op0=AL.add, op1=AL.mult)
                    y = dst if last else y
                if final_mul is not None:
                    dst = out if out is not None else pool.tile(
                        shp, FP, name=f"fm_{tag}", tag=f"y{tag}", bufs=bufs)
                    eng.tensor_mul(dst, y, final_mul)
                    y = dst
                return y

            def projection(src_tiles, w_tiles, es2, esb, asm_pool, ps_pool, tiny,
                           m, neg_time, v_mode=False, pfx=""):
                """One 128-row tile of a q/k/v-style lorentz projection.
                Assembled per-head layout: [time, sp64] for q/k, [sp64, time]
                for v (so the ave^T PSUM slices start at partitions 0/64)."""
                ps0 = ps_pool.tile([128, 512], FP, name="ps0", tag="prj", bufs=3)
                ps1 = ps_pool.tile([128, 512], FP, name="ps1", tag="prj", bufs=3)
                pst = ps_pool.tile([128, 1], FP, name="pst", tag="prjt", bufs=2)
                r0 = m * 128
                n = len(DCH)
                for i in range(n):
                    lhs = src_tiles[i][:, r0:r0 + 128]
                    st, sp = (i == 0), (i == n - 1)
                    nc.tensor.matmul(ps0, lhs, w_tiles[i][:, 0:512], start=st, stop=sp)
                    nc.tensor.matmul(ps1, lhs, w_tiles[i][:, 512:1024], start=st, stop=sp)
                    nc.tensor.matmul(pst, lhs, w_tiles[i][:, 1024:1025], start=st, stop=sp)

                sq0 = tiny.tile([128, 512], FP, name="sq0", tag="psq")
                sq1 = tiny.tile([128, 512], FP, name="sq1", tag="psq")
                s2p0 = tiny.tile([128, 1], FP, name="s2p0", tag="t_s2p0", bufs=2)
                s2p1 = tiny.tile([128, 1], FP, name="s2p1", tag="t_s2p1", bufs=2)
                nc.scalar.activation(sq0, ps0, AF.Square, accum_out=s2p0)
                nc.scalar.activation(sq1, ps1, AF.Square, accum_out=s2p1)
                nh = tiny.tile([128, 16], FP, name="nh", tag="t_nh", bufs=2)
                nc.vector.tensor_reduce(
                    nh[:, 0:8], sq0.rearrange("p (h d) -> p h d", d=64),
                    axis=mybir.AxisListType.X, op=AL.add)
                nc.vector.tensor_reduce(
                    nh[:, 8:16], sq1.rearrange("p (h d) -> p h d", d=64),
                    axis=mybir.AxisListType.X, op=AL.add)
                # time: sigmoid via tanh (stays in the exp table set)
                tnh = tiny.tile([128, 1], FP, name="tnh", tag="t_tnh", bufs=2)
                nc.scalar.activation(tnh, pst, AF.Tanh, scale=0.5)
                tm = tiny.tile([128, 1], FP, name="tm", tag="t_tm", bufs=2)
                nc.vector.tensor_scalar(tm, tnh, es2, esb, op0=AL.mult, op1=AL.add)
                tm2 = tiny.tile([128, 1], FP, name="tm2", tag="t_tm2", bufs=2)
                nc.vector.tensor_scalar(tm2, tm, tm, -1.0, op0=AL.mult, op1=AL.add)
                s2 = tiny.tile([128, 1], FP, name="s2", tag="t_s2", bufs=2)
                nc.vector.tensor_add(s2, s2p0, s2p1)
                prod = tiny.tile([128, 1], FP, name="prod", tag="t_prod", bufs=2)
                nc.vector.scalar_tensor_tensor(prod, s2, EPS, tm2,
                                               op0=AL.max, op1=AL.mult)
                # al = sqrt(tm2/s2) = tm2 * rsqrt(tm2*s2)
                al = rsqrt(tiny, prod, "al", newton=1, final_mul=tm2)
                a2 = tiny.tile([128, 1], FP, name="a2", tag="t_a2", bufs=2)
                nc.vector.tensor_mul(a2, al, al)
                thp = tiny.tile([128, 16], FP, name="thp", tag="t_thp", bufs=2)
                nc.vector.tensor_scalar(thp, nh, a2, 1.0, op0=AL.mult, op1=AL.add)
                th = rsqrt(tiny, thp, "th", newton=1, final_mul=thp)

                asm = asm_pool.tile([128, HP], DT_ATTN, name="asm", tag=f"{pfx}asm{m}")
                toff, soff = (64, 0) if v_mode else (0, 1)
                tdst = asm.rearrange("p (h d) -> p h d", d=65)[:, :, toff]
                nc.vector.tensor_scalar(
                    tdst, th, -1.0 if neg_time else 1.0, None, op0=AL.mult)
                sp_dst = asm.rearrange("p (h d) -> p h d", d=65)
                # split the two space-scaling writes across ScalarE and VectorE
                nc.scalar.activation(
                    sp_dst[:, 0:8, soff:soff + 64],
                    ps0.rearrange("p (h d) -> p h d", d=64), AF.Copy, scale=al)
                nc.scalar.activation(
                    sp_dst[:, 8:16, soff:soff + 64],
                    ps1.rearrange("p (h d) -> p h d", d=64), AF.Copy, scale=al)
                return asm

            # ---------------- phase 1+2: projections (K -> V -> Q) --------------
            # All input DMAs are issued up front (own tags, no aliasing) so the
            # in-order DMA queue never blocks the PE; the K/V exchanges are two
            # separate AllGathers so the kT gather starts ~40us earlier.
            RG = [[0, 1], [2, 3], [4, 5], [6, 7]]
            with tc.tile_pool(name="p1", bufs=1) as p1, \
                 tc.tile_pool(name="p1ps", bufs=1, space="PSUM") as p1ps, \
                 tc.tile_pool(name="p1tiny", bufs=3) as p1tiny:
                # interleave per-chunk so the first k matmuls start ASAP
                yts, wk = [], []
                for i, (d0, dn) in enumerate(DCH):
                    t = p1.tile([dn, RPC], DT_PROJ, name=f"yc{i}", tag=f"yc{i}")
                    nc.sync.dma_start(out=t, in_=yT[d0:d0 + dn, :])
                    yts.append(t)
                    w = p1.tile([dn, D], DT_PROJ, name=f"wk{i}", tag=f"wk{i}")
                    nc.sync.dma_start(out=w, in_=WkA[d0:d0 + dn, :])
                    wk.append(w)
                wv = load_w(p1, WvA, "wv")
                xts = load_chunks(p1, xT, "xc")
                wq = load_w(p1, WqA, "wq")
                k_asm = [projection(yts, wk, es2_sb["k"], esb_sb["k"], p1, p1ps,
                                    p1tiny, m, False, pfx="k") for m in range(4)]
                for h in range(16):
                    kt = p1.tile([65, RPC], DT_ATTN, name=f"kts{h}", tag="kts",
                                 bufs=4)
                    for m in range(4):
                        pst = p1ps.tile([65, 128], DT_ATTN, name="pstT", tag="tT", bufs=3)
                        nc.tensor.transpose(pst, k_asm[m][:, 65 * h:65 * h + 65], identity)
                        dst = kt[:, 128 * m:128 * m + 128]
                        if (4 * h + m) % 2:
                            nc.scalar.activation(dst, pst, AF.Copy)
                        else:
                            nc.vector.tensor_copy(dst, pst)
                    x, hh = divmod(h, 8)
                    nc.sync.dma_start(out=kT_sends[x][65 * hh:65 * hh + 65, :],
                                      in_=kt)
                    if h == 7 or h == 15:
                        # gather each 8-head half as soon as it is sent
                        nc.gpsimd.collective_compute(
                            "AllGather", AL.bypass, replica_groups=RG,
                            ins=[k_send_h[h // 8].opt()],
                            outs=[k_gath_h[h // 8].opt()],
                        )
                for m in range(4):
                    v_asm = projection(yts, wv, es2_sb["v"], esb_sb["v"], p1,
                                       p1ps, p1tiny, m, False, v_mode=True,
                                       pfx="v")
                    nc.sync.dma_start(
                        out=v_send[128 * m:128 * m + 128, :], in_=v_asm)
                nc.gpsimd.collective_compute(
                    "AllGather", AL.bypass, replica_groups=RG,
                    ins=[v_send_d.opt()], outs=[v_gath_d.opt()],
                )
                q_asm = [projection(xts, wq, es2_sb["q"], esb_sb["q"], p1, p1ps,
                                    p1tiny, m, True, pfx="q") for m in range(4)]
                # m-outer: transposes of m start as soon as q_asm[m] is ready
                for m in range(4):
                    for h in range(16):
                        pst = p1ps.tile([65, 128], DT_ATTN, name="pstQ", tag="tT", bufs=3)
                        nc.tensor.transpose(pst, q_asm[m][:, 65 * h:65 * h + 65], identity)
                        dst = qT[h][:, 128 * m:128 * m + 128]
                        if (4 * m + h) % 2:
                            nc.scalar.activation(dst, pst, AF.Copy)
                        else:
                            nc.vector.tensor_copy(dst, pst)

            # ---------------- phase 3: attention, head-pipelined ----------------
            with tc.tile_pool(name="p3", bufs=1) as p3, \
                 tc.tile_pool(name="p3ps", bufs=1, space="PSUM") as p3ps, \
                 tc.tile_pool(name="p3tiny", bufs=4) as p3tiny:
                # preload all gathered kT heads (ready first), then V, then the
                # phase-4 weights — all overlap the tail of the projections.
                # staging runs on two independent HW DMA queues (SP + GpSimd)
                kThs = []
                for h in range(16):
                    x, hh = divmod(h, 8)
                    kTh = p3.tile([65, T], DT_ATTN, name=f"kTh{h}", tag=f"kTh{h}")
                    nc.sync.dma_start(out=kTh[:, 0:512],
                                      in_=gath_kT(x, 0)[65 * hh:65 * hh + 65, :])
                    nc.scalar.dma_start(out=kTh[:, 512:1024],
                                        in_=gath_kT(x, 1)[65 * hh:65 * hh + 65, :])
                    kThs.append(kTh)
                vfull = []
                for i in range(8):
                    j, mm = divmod(i, 4)
                    vt = p3.tile([128, HP], DT_ATTN, name=f"vf{i}", tag=f"vf{i}")
                    q_eng = nc.scalar if i % 2 else nc.sync
                    q_eng.dma_start(
                        out=vt, in_=gath_v(j)[128 * mm:128 * mm + 128, :])
                    vfull.append(vt)
                wo = load_w(persist, WoA, "wo")

                pts = {}      # head -> [4 pt supertiles]
                zraws = {}    # head -> zraw tile
                invs = {}     # head -> inv [1,512] bf16

                def emit_scores(h):
                    pl = []
                    for s in range(4):
                        psS = p3ps.tile([128, 1024], FP, name="psS", tag="psS", bufs=2)
                        for half in range(2):
                            c = 2 * s + half
                            nc.tensor.matmul(psS[:, 512 * half:512 * half + 512],
                                             kThs[h][:, 128 * c:128 * c + 128], qT[h],
                                             start=True, stop=True)
                        pt = p3.tile([128, 1024], DT_ATTN, name="pt", tag="pt", bufs=8)
                        nc.scalar.activation(pt, psS, AF.Exp, bias=c025, scale=0.25)
                        pl.append(pt)
                    pts[h] = pl

                def emit_tail(g):
                    """AV + stats for head g (scores already issued)."""
                    psA = p3ps.tile([65, 512], FP, name="psA", tag="psA", bufs=1)
                    for i in range(8):
                        nc.tensor.matmul(psA, vfull[i][:, 65 * g:65 * g + 65],
                                         pts[g][i // 2][:, 512 * (i % 2):512 * (i % 2) + 512],
                                         start=(i == 0), stop=(i == 7))
                    zraw = p3.tile([65, 512], DT_ATTN, name="zraw", tag="zraw", bufs=4)
                    nc.scalar.activation(zraw, psA, AF.Copy)
                    zraws[g] = zraw
                    sq = p3.tile([65, 512], DT_ATTN, name="sqa", tag="sqa", bufs=2)
                    nc.vector.tensor_mul(sq, zraw, zraw)
                    psL = p3ps.tile([1, 512], FP, name="psL", tag="psL", bufs=2)
                    # sgn65 = [+1 x64, -1]: linner = sum(space^2) - time^2
                    nc.tensor.matmul(psL, sgn_sb, sq, start=True, stop=True)
                    ab = p3tiny.tile([1, 512], FP, name="ab", tag="ab")
                    # |L| via sign-bit clear; <ave,ave> is bounded away from 0
                    # here (timelike centroid) so no EPS clamp is needed.
                    nc.vector.tensor_scalar(ab.bitcast(I32), psL.bitcast(I32),
                                            0x7FFFFFFF, None, op0=AL.bitwise_and)
                    inv = p3.tile([1, 512], BF, name="inv", tag="inv", bufs=4)
                    rsqrt(p3tiny, ab, "inv", newton=1, out=inv)
                    invs[g] = inv
                    del pts[g]
                    if g % 2 == 1:
                        jz = g // 2
                        psB = p3ps.tile([128, 512], FP, name="psB", tag="psB", bufs=1)
                        nc.tensor.matmul(psB[0:64, :], onesr[:, 0:64],
                                         invs[g - 1], start=True, stop=True)
                        nc.tensor.matmul(psB[64:128, :], onesr[:, 0:64],
                                         invs[g], start=True, stop=True)
                        nc.vector.tensor_mul(zT[jz][0:64, :],
                                             zraws[g - 1][0:64, :], psB[0:64, :])
                        nc.vector.tensor_mul(zT[jz][64:128, :],
                                             zraws[g][0:64, :], psB[64:128, :])
                        nc.vector.tensor_mul(zsq[jz], zT[jz], zT[jz])
                        del zraws[g - 1], zraws[g], invs[g - 1], invs[g]

                for h in range(H + 1):
                    if h < H:
                        emit_scores(h)
                    if h >= 1:
                        emit_tail(h - 1)

                # global time: z8[0] = sqrt(1 + sum_h ||cent_h||^2)
                psG = p3ps.tile([1, 512], FP, name="psG", tag="psL", bufs=2)
                for j in range(8):
                    nc.tensor.matmul(psG, o128_sb, zsq[j],
                                     start=(j == 0), stop=(j == 7))
                sg = p3tiny.tile([1, 512], FP, name="sg", tag="ab")
                nc.vector.tensor_scalar(sg, psG, 1.0, None, op0=AL.add)
                rsqrt(p3tiny, sg, "zt", newton=2, out=z8[0:1, :], final_mul=sg)

            # ---------------- phase 4: output projection ------------------------
            with tc.tile_pool(name="p4", bufs=1) as p4, \
                 tc.tile_pool(name="p4ps", bufs=1, space="PSUM") as p4ps, \
                 tc.tile_pool(name="p4tiny", bufs=4) as p4tiny:
                zch = zT + [z8]
                g_sb, gq_sb = [], []
                ps2 = p4ps.tile([1, 512], FP, name="ps2", tag="ps2", bufs=1)
                for ct, (c0, cn) in enumerate(CCH):
                    psO = p4ps.tile([cn, 512], FP, name="psO", tag="psO", bufs=3)
                    # z8 (i=8) first: its wait on the global-time chain is paid
                    # once at the head of the phase instead of once per chunk
                    for i in [8] + list(range(8)):
                        nc.tensor.matmul(psO, wo[i][:, c0:c0 + cn], zch[i],
                                         start=(i == 8), stop=(i == 7))
                    g = p4.tile([cn, 512], FP, name="g", tag=f"g{ct}")
                    nc.scalar.activation(g, psO, AF.Copy)
                    gq = p4.tile([cn, 512], BF, name="gq", tag=f"gsq{ct}")
                    nc.scalar.activation(gq, psO, AF.Square)
                    g_sb.append(g)
                    gq_sb.append(gq)
                # ps2 accumulation deferred so the 81 psO matmuls run dense
                for ct, (c0, cn) in enumerate(CCH):
                    lhs_ones = o0_sb if ct == 0 else o128_sb[0:cn, :]
                    nc.tensor.matmul(ps2, lhs_ones, gq_sb[ct],
                                     start=(ct == 0), stop=(ct == 8))

                tnh = p4tiny.tile([1, 512], FP, name="tnho", tag="r4")
                nc.scalar.activation(tnh, g_sb[0][0:1, :], AF.Tanh, scale=0.5)
                to = p4tiny.tile([1, 512], FP, name="to", tag="r4")
                nc.vector.tensor_scalar(to, tnh, es2_sb["o"][0:1, :],
                                        esb_sb["o"][0:1, :], op0=AL.mult, op1=AL.add)
                t2 = p4tiny.tile([1, 512], FP, name="t2o", tag="r4")
                nc.vector.tensor_mul(t2, to, to)
                nc.vector.tensor_scalar_add(t2, t2, -1.0)
                prod = p4tiny.tile([1, 512], FP, name="prodo", tag="r4")
                nc.vector.scalar_tensor_tensor(prod, ps2, EPS, t2,
                                               op0=AL.max, op1=AL.mult)
                alo = p4.tile([1, 512], BF, name="alo", tag="alo")
                rsqrt(p4tiny, prod, "alo", newton=1, out=alo, final_mul=t2)
                psB4 = p4ps.tile([128, 512], FP, name="psB4", tag="psB4", bufs=1)
                nc.tensor.matmul(psB4, onesr, alo, start=True, stop=True)

                for ct, (c0, cn) in enumerate(CCH):
                    ot = p4.tile([cn, 512], FP, name="ot", tag="ot", bufs=3)
                    nc.vector.tensor_mul(ot, g_sb[ct], psB4[0:cn, :])
                    if ct == 0:
                        # overwrite the scaled row 0 with the recomputed time
                        nc.vector.tensor_copy(ot[0:1, :], to)
                    nc.sync.dma_start(out=outT[c0:c0 + cn, :], in_=ot)

    _split_excess_waits(nc)
    return nc


def _prep_w_r(W, b, dt):
    W_r = np.concatenate([W[1:], W[:1]], axis=0)
    b_r = np.concatenate([b[1:], b[:1]], axis=0)
    return np.ascontiguousarray(
        np.concatenate([W_r.T, b_r[None, :]], axis=0)).astype(dt)


def _prep_wo(Wo, bo, dt):
    return np.ascontiguousarray(
        np.concatenate([Wo[:, 1:].T, Wo[:, 0:1].T, bo[None, :]], axis=0)).astype(dt)


_NC_CACHE = None


def kernel(x, y, Wq, bq, sq, Wk, bk, sk, Wv, bv, sv, Wo, bo, so, attn_bias,
           _trace=False):
    global _NC_CACHE
    x = np.asarray(x, np.float32)
    y = np.asarray(y, np.float32)
    dt_proj = _np_dt(DT_PROJ)
    dt_bf = _np_dt(BF)

    WqA = _prep_w_r(np.asarray(Wq, np.float32), np.asarray(bq, np.float32), dt_proj)
    WkA = _prep_w_r(np.asarray(Wk, np.float32), np.asarray(bk, np.float32), dt_proj)
    WvA = _prep_w_r(np.asarray(Wv, np.float32), np.asarray(bv, np.float32), dt_proj)
    WoA = _prep_wo(np.asarray(Wo, np.float32), np.asarray(bo, np.float32), dt_proj)
    xT = np.concatenate([x.reshape(-1, D).T,
                         np.ones((1, B * T), np.float32)], axis=0).astype(dt_proj)
    yT = np.concatenate([y.reshape(-1, D).T,
                         np.ones((1, B * T), np.float32)], axis=0).astype(dt_proj)
    es = {nm: np.full((128, 1), np.exp(np.float32(s)), np.float32)
          for nm, s in (("esq", sq), ("esk", sk), ("esv", sv), ("eso", so))}
    sgn65 = np.concatenate([np.ones((64, 1)), -np.ones((1, 1))]).astype(dt_bf)
    ones128 = np.ones((128, 1), dt_bf)
    ones0 = np.ones((128, 1), dt_bf)
    ones0[0, 0] = 0

    if _NC_CACHE is None:
        _NC_CACHE = build_nc()
    nc = _NC_CACHE

    in_maps = []
    for c in range(N_CORES):
        rows = slice(RPC * c, RPC * (c + 1))
        in_maps.append(dict(
            xT=np.ascontiguousarray(xT[:, rows]),
            yT=np.ascontiguousarray(yT[:, rows]),
            WqA=WqA, WkA=WkA, WvA=WvA, WoA=WoA,
            esq=es["esq"], esk=es["esk"], esv=es["esv"], eso=es["eso"],
            sgn65=sgn65, ones128=ones128, ones0=ones0,
        ))

    res = run_bass_kernel_spmd(nc, in_maps, list(range(N_CORES)), trace=_trace)
    out = np.concatenate([res.results[c]["outT"].T for c in range(N_CORES)], axis=0)
    out = np.ascontiguousarray(out.astype(np.float32).reshape(B, T, D))
    if _trace:
        return out, res
    return out
